# revision 16
# baseline (speedup 1.0000x reference)
"""DAG-LSTM Trainium2 kernel.

Problem: 2-layer LSTM scanned over a 48-node DAG, batch 1024, hidden 256.
Sharding: pure data parallelism -- batch split 8 x 128 across NeuronCores,
weights replicated, no cross-device traffic.

Key optimization: the reference returns only the top-layer hidden state of
the LAST DAG node, so only the ancestor cone of (node 47, layer 1) needs to
be computed.  For the given pred graph that is 20 of the 96 (node, layer)
units -- a ~5x reduction in all engine work.  The cone is scheduled into
ASAP stages; units of the same (stage, layer) form groups of <= 2 so each
matmul's moving operand is u*128 columns.

Layout: "transposed" (feature-on-partition).  States h/c are [H=2x128
partition-chunks, B=128 free] fp16.  Gates for a group live in ONE
contiguous PSUM tile [128, 8 chunks, u*128] pre-loaded with the bias via
DMA, so all matmuls run with start=False (accumulate onto bias) and the
activations merge across chunks: sigmoid(i,f) / tanh(g) / sigmoid(o) are
three instructions per group instead of eight.  c-path stays fp16 (5e-4
rounding, verified against the fp32 oracle at 1.3e-3 total).
"""

import sys
import numpy as np

sys.path.insert(0, "/opt/trn_rl_repo")

B, N, IN, H, L, P = 1024, 48, 256, 256, 2, 2
NCORES = 8
BL = B // NCORES          # 128 batch per core
KC = 2                    # K chunks (256 = 2*128)
GROUP_MAX = 2             # units per (stage, layer) group

_CACHE = {}


def _unit_deps(pred, i, l):
    d = [(int(v) - 1, l) for v in pred[i] if v > 0]
    if l == 1:
        d.append((i, 0))
    return d


def _build_schedule(pred):
    """Ancestor cone of (N-1, 1) scheduled into ASAP stages; same-stage
    same-layer units grouped up to GROUP_MAX.  Returns list of
    (layer, [nodes]) in dependency order."""
    cone = set()
    stack = [(N - 1, 1)]
    while stack:
        u = stack.pop()
        if u in cone:
            continue
        cone.add(u)
        stack.extend(_unit_deps(pred, *u))
    stage = {}
    for u in sorted(cone):
        ds = [d for d in _unit_deps(pred, *u) if d in cone]
        stage[u] = max([stage[d] for d in ds], default=-1) + 1
    nstages = max(stage.values()) + 1
    groups = []
    for s in range(nstages):
        for l in (0, 1):
            nodes = sorted(i for (i, ll), st in stage.items()
                           if st == s and ll == l)
            for k in range(0, len(nodes), GROUP_MAX):
                groups.append((l, nodes[k:k + GROUP_MAX]))
    return groups


def _prep_weights(w_ih, w_hh, l):
    """Host-side weight prep -> [128, KC, 1024] fp16 stationary tiles.
    Layer-1 x input is h (fp16 states); the 0.5 predecessor-mean is folded
    into W_hh."""
    wx = w_ih
    wh = w_hh * 0.5

    def to_t(w):
        kdim = w.shape[1]
        wt = np.ascontiguousarray(w.T)            # [K, 1024]
        wt = wt.reshape(kdim // 128, 128, 1024)   # [kc, kin, 1024]
        return np.ascontiguousarray(
            wt.transpose(1, 0, 2).astype(np.float16))  # [128, kc, 1024]

    return to_t(wx), to_t(wh)


def _build_program(pred):
    from contextlib import ExitStack
    from concourse import bacc, mybir, tile

    f32 = mybir.dt.float32
    f16 = mybir.dt.float16
    AF = mybir.ActivationFunctionType
    Alu = mybir.AluOpType

    groups = _build_schedule(pred)
    l0_nodes = sorted({i for (l, nodes) in groups if l == 0 for i in nodes})
    l0_slot = {i: k for k, i in enumerate(l0_nodes)}

    nc = bacc.Bacc("TRN2", target_bir_lowering=False, debug=False,
                   num_devices=NCORES)

    dags_t = nc.dram_tensor("dags_t", [len(l0_nodes), 128, KC, 128], f16,
                            kind="ExternalInput")
    h0_t = nc.dram_tensor("h0_t", [128, L, KC, 128], f16,
                          kind="ExternalInput")
    c0_t = nc.dram_tensor("c0_t", [128, L, KC, 128], f32,
                          kind="ExternalInput")
    w_dram = {}
    for l in range(L):
        w_dram[("x", l)] = nc.dram_tensor(f"wx{l}", [128, KC, 1024], f16,
                                          kind="ExternalInput")
        w_dram[("h", l)] = nc.dram_tensor(f"wh{l}", [128, KC, 1024], f16,
                                          kind="ExternalInput")
    # bias image pre-broadcast for PSUM preload: [128, L, 8, GROUP_MAX*128]
    bias_dram = nc.dram_tensor("bias_img", [128, L, 8, GROUP_MAX * 128], f16,
                               kind="ExternalInput")
    out_t = nc.dram_tensor("out_t", [KC, 128, 128], f32, kind="ExternalOutput")

    with tile.TileContext(nc) as tc, ExitStack() as ctx:
        consts = ctx.enter_context(tc.tile_pool(name="consts", bufs=1))
        ps = ctx.enter_context(tc.tile_pool(name="ps", bufs=2, space="PSUM"))
        gp = ctx.enter_context(tc.tile_pool(name="gp", bufs=2))

        # initial states + bias image + weights
        slot0_h = consts.tile([128, L, KC, 128], f16, tag="slot0h")
        nc.sync.dma_start(out=slot0_h[:], in_=h0_t[:])
        slot0_c = consts.tile([128, L, KC, 128], f32, tag="slot0c")
        nc.sync.dma_start(out=slot0_c[:], in_=c0_t[:])
        bias_sb = consts.tile([128, L, 8, GROUP_MAX * 128], f16, tag="bias")
        nc.sync.dma_start(out=bias_sb[:], in_=bias_dram[:])
        wsb = {}
        for key in [("x", 0), ("h", 0), ("x", 1), ("h", 1)]:
            t = consts.tile([128, KC, 1024], f16, tag=f"w{key[0]}{key[1]}",
                            name=f"w{key[0]}{key[1]}")
            nc.gpsimd.dma_start(out=t[:], in_=w_dram[key][:])
            wsb[key] = t
        # per-group persistent state tiles; units are views [:, j]
        st_h = {}
        st_c = {}
        for g, (l, nodes) in enumerate(groups):
            u = len(nodes)
            gh = consts.tile([128, u, KC, 128], f16, tag=f"gh{g}",
                             name=f"gh{g}")
            gc = consts.tile([128, u, KC, 128], f32, tag=f"gc{g}",
                             name=f"gc{g}")
            for j, i in enumerate(nodes):
                st_h[(i, l)] = gh[:, j]
                st_c[(i, l)] = gc[:, j]
            groups[g] = (l, nodes, gh, gc)
        outh = consts.tile([128, KC, 128], f32, tag="outh")

        def h_ap(v, l):
            if v == 0:
                return slot0_h[:, l]
            return st_h[(v - 1, l)]

        def c_ap(v, l):
            if v == 0:
                return slot0_c[:, l]
            return st_c[(v - 1, l)]

        for (l, nodes, gh, gc) in groups:
            u = len(nodes)
            un = u * 128
            # one contiguous psum tile for all 8 gate chunks
            pt = ps.tile([128, 8, GROUP_MAX * 128], f32, tag="gates",
                         name="gates")
            # bias preload (all matmuls then accumulate with start=False)
            nc.vector.tensor_copy(out=pt[:, :, :un], in_=bias_sb[:, l, :, :un])

            xq = gp.tile([128, KC, u, 128], f16, tag="xq", name="xq")
            ubh = gp.tile([128, KC, u, 128], f16, tag="ubh")
            ubc = gp.tile([128, KC, u, 128], f32, tag="ubc")
            sifo = gp.tile([128, 4, u, 128], f16, tag="sifo")
            gt = gp.tile([128, KC, u, 128], f16, tag="gt")
            so = gp.tile([128, KC, u, 128], f16, tag="so")
            vw = gp.tile([128, KC, u, 128], f16, tag="vw")
            cf = gp.tile([128, KC, u, 128], f32, tag="cf")
            th = gp.tile([128, u, KC, 128], f16, tag="th")

            # 1. inputs: x (layer0: DMA; layer1: Pool copy of h_l0)
            for j, i in enumerate(nodes):
                if l == 0:
                    nc.sync.dma_start(out=xq[:, :, j, :],
                                      in_=dags_t[l0_slot[i]])
                else:
                    nc.vector.tensor_copy(out=xq[:, :, j, :],
                                          in_=st_h[(i, 0)])

            # 2. predecessor state sums (h on DVE, c on Pool)
            for j, i in enumerate(nodes):
                a, b_ = int(pred[i][0]), int(pred[i][1])
                if a == b_:
                    nc.vector.tensor_scalar_mul(ubh[:, :, j, :], h_ap(a, l),
                                                2.0)
                    nc.vector.tensor_scalar_mul(ubc[:, :, j, :], c_ap(a, l),
                                                2.0)
                else:
                    nc.vector.tensor_tensor(out=ubh[:, :, j, :],
                                            in0=h_ap(a, l), in1=h_ap(b_, l),
                                            op=Alu.add)
                    nc.vector.tensor_tensor(out=ubc[:, :, j, :],
                                            in0=c_ap(a, l), in1=c_ap(b_, l),
                                            op=Alu.add)

            # 3. gates: accumulate onto the bias preload, start=False.
            # Chunk order i,f,g first (c-path critical), o last.
            for m in range(8):
                ops = [("x", xq), ("h", ubh)] if l == 0 else \
                      [("h", ubh), ("x", xq)]
                last = False
                for oi, (kind, src) in enumerate(ops):
                    for k in range(KC):
                        last = (oi == 1 and k == KC - 1)
                        nc.tensor.matmul(
                            out=pt[:, m, :un],
                            lhsT=wsb[(kind, l)][:, k, m * 128:(m + 1) * 128],
                            rhs=src[:, k].rearrange("p u b -> p (u b)"),
                            start=False, stop=last, skip_group_check=True)

            # 4. merged activations: sigmoid(i,f), tanh(g), sigmoid(o)
            nc.scalar.activation(out=sifo[:].rearrange("p c u b -> p c (u b)"),
                                 in_=pt[:, 0:4, :un], func=AF.Sigmoid)
            nc.scalar.activation(out=gt[:].rearrange("p c u b -> p c (u b)"),
                                 in_=pt[:, 4:6, :un], func=AF.Tanh)
            nc.scalar.activation(out=so[:].rearrange("p c u b -> p c (u b)"),
                                 in_=pt[:, 6:8, :un], func=AF.Sigmoid)

            # 5. c_new = (ubc*0.5)*sigmoid(f) + sigmoid(i)*tanh(g)  [Pool]
            nc.vector.tensor_tensor(out=vw[:], in0=sifo[:, 0:2], in1=gt[:],
                                    op=Alu.mult)
            nc.vector.scalar_tensor_tensor(
                out=cf[:], in0=ubc[:], scalar=0.5, in1=sifo[:, 2:4],
                op0=Alu.mult, op1=Alu.mult)
            nc.vector.tensor_tensor(out=gc[:].rearrange("p u c b -> p c u b"),
                                    in0=cf[:], in1=vw[:], op=Alu.add)

            # 6. h = sigmoid(o) * tanh(c)
            nc.scalar.activation(out=th[:].rearrange("p u c b -> p (u c b)"),
                                 in_=gc[:].rearrange("p u c b -> p (u c b)"),
                                 func=AF.Tanh)
            nc.vector.tensor_tensor(out=gh[:].rearrange("p u c b -> p c u b"),
                                    in0=so[:],
                                    in1=th[:].rearrange("p u c b -> p c u b"),
                                    op=Alu.mult)
            if l == 1 and N - 1 in nodes:
                j = nodes.index(N - 1)
                nc.vector.tensor_tensor(out=outh[:], in0=so[:, :, j, :],
                                        in1=th[:, j], op=Alu.mult)

        # output: h of last node, top layer: [128, KC, 128] -> [KC, 128, 128]
        nc.sync.dma_start(out=out_t.ap().rearrange("k p b -> p k b"),
                          in_=outh[:])

    nc.compile()
    return nc, l0_nodes


def _get_program(pred):
    key = pred.tobytes()
    if key not in _CACHE:
        _CACHE[key] = _build_program(pred)
    return _CACHE[key]


def _prepare(dags, h0, c0, w_ih0, w_hh0, b_ih0, b_hh0,
             w_ih1, w_hh1, b_ih1, b_hh1, pred_idx):
    """Host-side prep: returns (nc, in_maps)."""
    dags = np.asarray(dags, dtype=np.float32)
    h0 = np.asarray(h0, dtype=np.float32)
    c0 = np.asarray(c0, dtype=np.float32)
    pred = np.asarray(pred_idx)

    nc, l0_nodes = _get_program(pred)

    wx0, wh0 = _prep_weights(np.asarray(w_ih0, np.float32),
                             np.asarray(w_hh0, np.float32), 0)
    wx1, wh1 = _prep_weights(np.asarray(w_ih1, np.float32),
                             np.asarray(w_hh1, np.float32), 1)
    bias = np.stack([
        np.asarray(b_ih0, np.float32) + np.asarray(b_hh0, np.float32),
        np.asarray(b_ih1, np.float32) + np.asarray(b_hh1, np.float32),
    ])  # [L, 1024]
    # bias image [128, L, 8, GROUP_MAX*128]: row (chunk*128 + p) broadcast
    bimg = np.ascontiguousarray(
        np.broadcast_to(
            bias.reshape(L, 8, 128).transpose(2, 0, 1)[:, :, :, None],
            (128, L, 8, GROUP_MAX * 128)).astype(np.float16))

    in_maps = []
    for c in range(NCORES):
        bs = slice(c * BL, (c + 1) * BL)
        # dags [B, N, IN] -> cone nodes only -> [n, kin(128), kc, b] fp16
        sel = dags[bs][:, l0_nodes]                      # [BL, n, IN]
        dt_ = sel.transpose(1, 2, 0).reshape(len(l0_nodes), KC, 128, BL)
        dt_ = np.ascontiguousarray(
            dt_.transpose(0, 2, 1, 3).astype(np.float16))
        # h0/c0 [L, B, H] -> [128(p), L, kc, b] fp16
        hh = h0[:, bs, :].transpose(2, 0, 1).reshape(KC, 128, L, BL)
        cc = c0[:, bs, :].transpose(2, 0, 1).reshape(KC, 128, L, BL)
        h0t = np.ascontiguousarray(
            hh.transpose(1, 2, 0, 3).astype(np.float16))  # [128, L, kc, b]
        c0t = np.ascontiguousarray(
            cc.transpose(1, 2, 0, 3).astype(np.float32))
        in_maps.append({
            "dags_t": dt_, "h0_t": h0t, "c0_t": c0t,
            "wx0": wx0, "wh0": wh0, "wx1": wx1, "wh1": wh1,
            "bias_img": bimg,
        })
    return nc, in_maps


def _assemble(res):
    out = np.empty((B, H), np.float32)
    for c in range(NCORES):
        ot = res.results[c]["out_t"]  # [KC, 128, 128] = [kc, p, b]
        out[c * BL:(c + 1) * BL] = ot.reshape(H, BL).T
    return out


def kernel(**inputs):
    from concourse.bass_utils import run_bass_kernel_spmd

    nc, in_maps = _prepare(**inputs)
    res = run_bass_kernel_spmd(nc, in_maps, list(range(NCORES)))
    return _assemble(res)
